# revision 18
# baseline (speedup 1.0000x reference)
"""DFH loss kernel for Trainium2, data-parallel across 8 NeuronCores.

Computation (see reference):
  U[:, ind] = u.T; Y[:, ind] = y.T                       (host, part of sharding prep)
  b = sign(C @ y.T + u.T), C = sign(V)                   (device prologue)
  V_new = 200 GD steps on V                              (device, redundant per core)
  metric = mean(softplus(M + (1-2s) * 0.5*(u@U)))        (device, sharded over num_train)
  quant  = mean((b - u.T)^2)
  loss = metric + ETA * quant

Sharding: U/Y split along num_train (12500 cols/core); everything else replicated.
Per-core partial sums are combined on the host (8 scalars).

Device-side math notes:
  s = y@Y is exactly {0,1} (one-hot x one-hot), so the two softplus branches
  collapse to softplus(M + r*ip) with r = 1-2s. r comes from one K=101 bf16
  matmul with lhsT = [-2*y.T; 1] and rhs = [Y; 1] (the +1 fold).
  The clip(ip, -100, 50) never binds for this data distribution (max |ip| < 50
  checked against the reference; softplus is evaluated with proper asymptotics
  on ACT, so no overflow concerns).

  Center_gradient is run in transposed form W = V.T [100, 64]:
    W <- kv ⊙ W + beta*(W @ (W.T W)) + 64*beta*(J @ W) + alpha*c3*S' + Bts
  with S' = 2*(W>0), kv/Bts/J folding all per-alpha constants.
"""

import sys
import numpy as np

for _p in ("/opt/trn_rl_repo", "/root/.axon_site/_ro/trn_rl_repo"):
    if _p not in sys.path:
        sys.path.append(_p)

import ml_dtypes

BF16 = ml_dtypes.bfloat16

BIT = 64
N_CLASS = 100
NUM_TRAIN = 100000
BATCH = 512
MU = 1.0
M = 1.0
ETA = 0.5
VUL = 1.0
NTA = 1.0

N_CORES = 8
NSH = NUM_TRAIN // N_CORES          # 12500 columns per core
CHUNK = 2500                        # DMA/ACT chunk (cols)
SUB = 500                           # matmul N (one PSUM bank)
N_CHUNKS = NSH // CHUNK
N_SUBS = CHUNK // SUB
N_BLK = BATCH // 128                # 4 batch blocks

C1 = 2.0 / (BIT * BATCH)
C2 = 4.0 / (N_CLASS * N_CLASS) * VUL
C3 = 2.0 / (BIT * N_CLASS) * NTA
ALPHAS = (0.03, 0.003, 0.0003)
N_ITERS = 200

TRACE = False           # set by test.py to capture an NTFF profile
LAST_RESULTS = None     # BassKernelResults of the most recent run
ENABLE_CG = True        # debug bisect toggles
ENABLE_METRIC = True
ENABLE_PROLOGUE = True
CG_ITERS = N_ITERS


def _alpha_idx(i):
    return 0 if i < 149 else (1 if i < 179 else 2)


def _build_program():
    import concourse.bacc as bacc
    import concourse.mybir as mybir
    from concourse import tile

    dt = mybir.dt
    f32 = dt.float32
    bf16 = dt.bfloat16
    Alu = mybir.AluOpType
    Act = mybir.ActivationFunctionType

    nc = bacc.Bacc("TRN2", target_bir_lowering=False, debug=False)

    # ---- DRAM I/O ----
    d_uhalf = nc.dram_tensor("u_half", [BIT, BATCH], bf16, kind="ExternalInput")
    d_ytaug = nc.dram_tensor("yt_aug", [N_CLASS + 1, BATCH], bf16, kind="ExternalInput")
    d_us = nc.dram_tensor("Us", [BIT, NSH], bf16, kind="ExternalInput")
    d_ys = nc.dram_tensor("Ys", [N_CLASS + 1, NSH], bf16, kind="ExternalInput")
    d_ytp = nc.dram_tensor("yt_plain", [N_CLASS, BATCH], f32, kind="ExternalInput")
    d_yblk = nc.dram_tensor("y_blocks", [128, N_BLK, N_CLASS], f32, kind="ExternalInput")
    d_ublk = nc.dram_tensor("u_blocks", [128, N_BLK, BIT], f32, kind="ExternalInput")
    d_vt = nc.dram_tensor("VT", [N_CLASS, BIT], f32, kind="ExternalInput")
    d_i100 = nc.dram_tensor("I100", [N_CLASS, N_CLASS], f32, kind="ExternalInput")
    d_j3 = nc.dram_tensor("J3", [N_CLASS, 3, N_CLASS], f32, kind="ExternalInput")

    d_partials = nc.dram_tensor("partials", [128, 2], f32, kind="ExternalOutput")
    d_vout = nc.dram_tensor("V_out", [BIT, N_CLASS], f32, kind="ExternalOutput")

    with tile.TileContext(nc) as tc:
        with (
            tc.tile_pool(name="cpool", bufs=1) as cpool,
            tc.tile_pool(name="wpool", bufs=2) as wpool,
            tc.tile_pool(name="mpsum", bufs=2, space="PSUM") as mpsum,
            tc.tile_pool(name="gpsum", bufs=1, space="PSUM") as gpsum,
        ):
            # ---- constants into SBUF ----
            c_uhalf = cpool.tile([BIT, BATCH], bf16, name="c_uhalf")
            c_ytaug = cpool.tile([N_CLASS + 1, BATCH], bf16, name="c_ytaug")
            c_ytp = cpool.tile([N_CLASS, BATCH], f32, name="c_ytp")
            c_yblk = cpool.tile([128, N_BLK, N_CLASS], f32, name="c_yblk")
            c_ublk = cpool.tile([128, N_BLK, BIT], f32, name="c_ublk")
            c_vt = cpool.tile([N_CLASS, BIT], f32, name="c_vt")
            c_i100 = cpool.tile([N_CLASS, N_CLASS], f32, name="c_i100")
            c_j3 = cpool.tile([N_CLASS, 3, N_CLASS], f32, name="c_j3")
            nc.sync.dma_start(c_uhalf[:], d_uhalf[:])
            nc.sync.dma_start(c_ytaug[:], d_ytaug[:])
            nc.sync.dma_start(c_ytp[:], d_ytp[:])
            nc.sync.dma_start(c_yblk[:], d_yblk[:])
            nc.sync.dma_start(c_ublk[:], d_ublk[:])
            nc.sync.dma_start(c_vt[:], d_vt[:])
            nc.sync.dma_start(c_i100[:], d_i100[:])
            nc.sync.dma_start(c_j3[:], d_j3[:])

            ones128 = cpool.tile([128, 1], f32, name="ones128")
            nc.vector.memset(ones128[:], 1.0)

            acc_m = cpool.tile([128, N_CHUNKS * N_BLK], f32, name="acc_m")
            acc_q = cpool.tile([128, N_BLK], f32, name="acc_q")
            nc.vector.memset(acc_m[:], 0.0)
            nc.vector.memset(acc_q[:], 0.0)
            bt_all = cpool.tile([128, N_BLK, BIT], f32, name="bt_all")
            ct = cpool.tile([N_CLASS, BIT], f32, name="ct")
            bts = cpool.tile([N_CLASS, 3, BIT], f32, name="bts")
            kv = cpool.tile([N_CLASS, 3], f32, name="kv")

            # ---- prologue: b, quant partials, Bt, counts ----
            nc.scalar.activation(ct[:], c_vt[:], Act.Sign)  # sign(V).T
            if not ENABLE_PROLOGUE:
                nc.vector.memset(bt_all[:], 0.0)
            for blk in range(N_BLK if ENABLE_PROLOGUE else 0):
                pb = gpsum.tile([128, BIT], f32, name="pb", tag="cgP")
                nc.tensor.matmul(
                    pb[:], c_ytp[:, blk * 128:(blk + 1) * 128], ct[:],
                    start=True, stop=True,
                )
                bpre = wpool.tile([128, BIT], f32, name="bpre", tag="bpre")
                nc.vector.tensor_tensor(bpre[:], pb[:], c_ublk[:, blk, :], Alu.add)
                nc.scalar.activation(bt_all[:, blk, :], bpre[:], Act.Sign)
                dq = wpool.tile([128, BIT], f32, name="dq", tag="dq")
                nc.vector.tensor_tensor(dq[:], bt_all[:, blk, :], c_ublk[:, blk, :], Alu.subtract)
                # (tensor_tensor_reduce crashes this runtime; Square+accum on ACT)
                dqj = wpool.tile([128, BIT], f32, name="dqj", tag="dqj")
                nc.scalar.activation(dqj[:], dq[:], Act.Square,
                                     accum_out=acc_q[:, blk:blk + 1])

            btp = gpsum.tile([N_CLASS, BIT], f32, name="btp", tag="cgB")
            for blk in range(N_BLK):
                nc.tensor.matmul(
                    btp[:], c_yblk[:, blk, :], bt_all[:, blk, :],
                    start=(blk == 0), stop=(blk == N_BLK - 1),
                )
            cntp = gpsum.tile([N_CLASS, 1], f32, name="cntp", tag="cgF")
            for blk in range(N_BLK):
                nc.tensor.matmul(
                    cntp[:], c_yblk[:, blk, :], ones128[:],
                    start=(blk == 0), stop=(blk == N_BLK - 1),
                )
            for a, alpha in enumerate(ALPHAS):
                nc.vector.tensor_scalar(
                    bts[:, a, :], btp[:], float(alpha * C1), float(-alpha * C3),
                    Alu.mult, Alu.add,
                )
                nc.vector.tensor_scalar(
                    kv[:, a:a + 1], cntp[:], float(-alpha * C1),
                    float(1.0 - alpha * C3 + 128 * alpha * C2),
                    Alu.mult, Alu.add,
                )

            # ---- center gradient: 200 iterations on W = V.T [100, 64] ----
            cg_scope = nc.named_scope("cg")
            cg_scope.__enter__()
            W = c_vt
            for i in range(CG_ITERS if ENABLE_CG else 0):
                a = _alpha_idx(i)
                alpha = ALPHAS[a]
                beta = -alpha * C2
                sp = wpool.tile([N_CLASS, BIT], f32, name="sp", tag="sp")
                nc.vector.tensor_scalar(
                    sp[:], W[:], 0.0, float(2 * alpha * C3), Alu.is_gt, Alu.mult,
                )
                x = wpool.tile([N_CLASS, BIT], f32, name="x", tag="x")
                nc.vector.tensor_tensor(x[:], sp[:], bts[:, a, :], Alu.add)

                fps = gpsum.tile([BIT, BIT], f32, name="fps", tag="cgF")
                nc.tensor.matmul(fps[:], W[:], W[:], start=True, stop=True)
                fb = wpool.tile([BIT, BIT], f32, name="fb", tag="fb")
                nc.vector.tensor_scalar(fb[:], fps[:], float(beta), None, Alu.mult)

                ptp = gpsum.tile([BIT, N_CLASS], f32, name="ptp", tag="cgT")
                nc.tensor.transpose(ptp[:], W[:], c_i100[:])
                vc = wpool.tile([BIT, N_CLASS], f32, name="vc", tag="vc")
                nc.vector.tensor_copy(vc[:], ptp[:])

                pp = gpsum.tile([N_CLASS, BIT], f32, name="pp", tag="cgP")
                nc.tensor.matmul(pp[:], vc[:], fb[:], start=True, stop=False)
                nc.tensor.matmul(pp[:], c_j3[:, a, :], W[:], start=False, stop=False)
                nc.tensor.matmul(pp[:], c_i100[:], x[:], start=False, stop=True)

                wn = wpool.tile([N_CLASS, BIT], f32, name="wn", tag="W")
                nc.vector.scalar_tensor_tensor(
                    out=wn[:], in0=W[:], scalar=kv[:, a:a + 1], in1=pp[:],
                    op0=Alu.mult, op1=Alu.add,
                )
                W = wn

            vtp = gpsum.tile([BIT, N_CLASS], f32, name="vtp", tag="cgT")
            nc.tensor.transpose(vtp[:], W[:], c_i100[:])
            vout = wpool.tile([BIT, N_CLASS], f32, name="vout", tag="vout")
            nc.vector.tensor_copy(vout[:], vtp[:])
            nc.sync.dma_start(d_vout[:], vout[:])
            cg_scope.__exit__(None, None, None)

            # ---- metric loss over this core's shard ----
            m_scope = nc.named_scope("metric")
            m_scope.__enter__()
            for k in range(N_CHUNKS if ENABLE_METRIC else 0):
                uc = wpool.tile([BIT, CHUNK], bf16, name="uc", tag="uc")
                nc.sync.dma_start(uc[:], d_us[:, k * CHUNK:(k + 1) * CHUNK])
                yc = wpool.tile([N_CLASS + 1, CHUNK], bf16, name="yc", tag="yc")
                nc.sync.dma_start(yc[:], d_ys[:, k * CHUNK:(k + 1) * CHUNK])
                for blk in range(N_BLK):
                    tbig = wpool.tile([128, CHUNK], f32, name="tbig", tag="t")
                    spo = wpool.tile([128, CHUNK], f32, name="spo", tag="spo")
                    for j in range(N_SUBS):
                        ipp = mpsum.tile([128, SUB], f32, name="ipp", tag="ip")
                        nc.tensor.matmul(
                            ipp[:], c_uhalf[:, blk * 128:(blk + 1) * 128],
                            uc[:, j * SUB:(j + 1) * SUB], start=True, stop=True,
                        )
                        rp = mpsum.tile([128, SUB], f32, name="rp", tag="r")
                        nc.tensor.matmul(
                            rp[:], c_ytaug[:, blk * 128:(blk + 1) * 128],
                            yc[:, j * SUB:(j + 1) * SUB], start=True, stop=True,
                        )
                        # DVE can read only one PSUM operand per instruction:
                        # stage r in SBUF, then multiply with ip from PSUM.
                        rsb = wpool.tile([128, SUB], f32, name="rsb", tag="rsb")
                        nc.vector.tensor_copy(rsb[:], rp[:])
                        nc.vector.tensor_tensor(
                            tbig[:, j * SUB:(j + 1) * SUB], ipp[:], rsb[:], Alu.mult,
                        )
                    # softplus(t) = ln(1 + e^t); t <= ~51 so e^t stays finite
                    # in fp32. Exp and Ln live in one ACT table set
                    # (natural_log_exp_and_others) -> no table switching.
                    col = k * N_BLK + blk
                    etile = wpool.tile([128, CHUNK], f32, name="etile", tag="et")
                    nc.scalar.activation(
                        etile[:], tbig[:], Act.Exp, bias=float(M), scale=1.0,
                    )
                    nc.scalar.activation(
                        spo[:], etile[:], Act.Ln, bias=1.0, scale=1.0,
                        accum_out=acc_m[:, col:col + 1],
                    )

            m_scope.__exit__(None, None, None)

            # ---- reductions + store ----
            pr = wpool.tile([128, 2], f32, name="pr", tag="pr")
            nc.vector.tensor_reduce(pr[:, 0:1], acc_m[:], mybir.AxisListType.X, Alu.add)
            nc.vector.tensor_reduce(pr[:, 1:2], acc_q[:], mybir.AxisListType.X, Alu.add)
            nc.sync.dma_start(d_partials[:], pr[:])

    nc.compile()
    return nc


def _host_inputs(u, y, ind, U, Y, V):
    """Scatter + shard + layout prep. Returns per-core input maps."""
    u = np.asarray(u, np.float32)
    y = np.asarray(y, np.float32)
    ind = np.asarray(ind)
    U2 = np.array(U, np.float32, copy=True)
    Y2 = np.array(Y, np.float32, copy=True)
    U2[:, ind] = u.T
    Y2[:, ind] = y.T

    u_half = np.ascontiguousarray(0.5 * u.T).astype(BF16)
    yt_aug = np.concatenate(
        [-2.0 * y.T, np.ones((1, BATCH), np.float32)], axis=0
    ).astype(BF16)
    yt_plain = np.ascontiguousarray(y.T)
    y_blocks = np.ascontiguousarray(
        y.reshape(N_BLK, 128, N_CLASS).transpose(1, 0, 2))
    u_blocks = np.ascontiguousarray(
        u.reshape(N_BLK, 128, BIT).transpose(1, 0, 2))
    VT = np.ascontiguousarray(np.asarray(V, np.float32).T)
    I100 = np.eye(N_CLASS, dtype=np.float32)
    J3 = np.stack(
        [np.full((N_CLASS, N_CLASS), 64.0 * (-a * C2), np.float32) for a in ALPHAS],
        axis=1,
    )
    ones_row = np.ones((1, NSH), np.float32)

    shared = dict(
        u_half=u_half, yt_aug=yt_aug, yt_plain=yt_plain, y_blocks=y_blocks,
        u_blocks=u_blocks, VT=VT, I100=I100, J3=J3,
    )
    in_maps = []
    for c in range(N_CORES):
        sl = slice(c * NSH, (c + 1) * NSH)
        in_maps.append(dict(
            shared,
            Us=np.ascontiguousarray(U2[:, sl]).astype(BF16),
            Ys=np.concatenate([Y2[:, sl], ones_row], axis=0).astype(BF16),
        ))
    return in_maps


_PROGRAM = None


def kernel(u, y, ind, U, Y, V):
    global _PROGRAM, LAST_RESULTS
    from concourse.bass_utils import run_bass_kernel_spmd

    in_maps = _host_inputs(u, y, ind, U, Y, V)
    if _PROGRAM is None:
        _PROGRAM = _build_program()
    res = run_bass_kernel_spmd(
        _PROGRAM, in_maps, core_ids=list(range(N_CORES)), trace=TRACE,
    )
    LAST_RESULTS = res

    msum = 0.0
    for c in range(N_CORES):
        msum += float(res.results[c]["partials"][:, 0].astype(np.float64).sum())
    qsum = float(res.results[0]["partials"][:, 1].astype(np.float64).sum())
    metric = msum / (BATCH * NUM_TRAIN)
    quant = qsum / (BIT * BATCH)
    loss = np.float32(metric + ETA * quant)
    V_new = np.asarray(res.results[0]["V_out"], np.float32)
    return loss, V_new


# revision 22
# speedup vs baseline: 323.4879x; 323.4879x over previous
"""DFH loss kernel for Trainium2, data-parallel across 8 NeuronCores.

Computation (see reference):
  U[:, ind] = u.T; Y[:, ind] = y.T                       (host, part of sharding prep)
  b = sign(C @ y.T + u.T), C = sign(V)                   (device prologue)
  V_new = 200 GD steps on V                              (device, redundant per core)
  metric = mean(softplus(M + (1-2s) * 0.5*(u@U)))        (device, sharded over num_train)
  quant  = mean((b - u.T)^2)
  loss = metric + ETA * quant

Sharding: U/Y split along num_train (12500 cols/core); everything else replicated.
Per-core partial sums are combined on the host (8 scalars).

Device-side math notes:
  s = y@Y is exactly {0,1} (one-hot x one-hot), so the two softplus branches
  collapse to softplus(M + r*ip) with r = 1-2s. r comes from one K=101 bf16
  matmul with lhsT = [-2*y.T; 1] and rhs = [Y; 1] (the +1 fold).
  The clip(ip, -100, 50) never binds for this data distribution (max |ip| < 50
  checked against the reference; softplus is evaluated with proper asymptotics
  on ACT, so no overflow concerns).

  Center_gradient is run in transposed form W = V.T [100, 64]:
    W <- kv ⊙ W + beta*(W @ (W.T W)) + 64*beta*(J @ W) + alpha*c3*S' + Bts
  with S' = 2*(W>0), kv/Bts/J folding all per-alpha constants.
"""

import sys
import numpy as np

for _p in ("/opt/trn_rl_repo", "/root/.axon_site/_ro/trn_rl_repo"):
    if _p not in sys.path:
        sys.path.append(_p)

import ml_dtypes

BF16 = ml_dtypes.bfloat16

BIT = 64
N_CLASS = 100
NUM_TRAIN = 100000
BATCH = 512
MU = 1.0
M = 1.0
ETA = 0.5
VUL = 1.0
NTA = 1.0

N_CORES = 8
NSH = NUM_TRAIN // N_CORES          # 12500 columns per core
CHUNK = 2500                        # DMA/ACT chunk (cols)
SUB = 500                           # matmul N (one PSUM bank)
N_CHUNKS = NSH // CHUNK
N_SUBS = CHUNK // SUB
N_BLK = BATCH // 128                # 4 batch blocks

C1 = 2.0 / (BIT * BATCH)
C2 = 4.0 / (N_CLASS * N_CLASS) * VUL
C3 = 2.0 / (BIT * N_CLASS) * NTA
ALPHAS = (0.03, 0.003, 0.0003)
N_ITERS = 200

TRACE = False           # set by test.py to capture an NTFF profile
LAST_RESULTS = None     # BassKernelResults of the most recent run
ENABLE_CG = True        # debug bisect toggles
ENABLE_METRIC = True
ENABLE_PROLOGUE = True
CG_ITERS = N_ITERS


def _alpha_idx(i):
    return 0 if i < 149 else (1 if i < 179 else 2)


def _build_program():
    import concourse.bacc as bacc
    import concourse.mybir as mybir
    from concourse import tile

    dt = mybir.dt
    f32 = dt.float32
    bf16 = dt.bfloat16
    Alu = mybir.AluOpType
    Act = mybir.ActivationFunctionType

    nc = bacc.Bacc("TRN2", target_bir_lowering=False, debug=False)

    # ---- DRAM I/O ----
    d_uhalf = nc.dram_tensor("u_half", [BIT, BATCH], bf16, kind="ExternalInput")
    d_ytaug = nc.dram_tensor("yt_aug", [N_CLASS + 1, BATCH], bf16, kind="ExternalInput")
    d_us = nc.dram_tensor("Us", [BIT, NSH], bf16, kind="ExternalInput")
    d_ys = nc.dram_tensor("Ys", [N_CLASS + 1, NSH], bf16, kind="ExternalInput")
    d_ytp = nc.dram_tensor("yt_plain", [N_CLASS, BATCH], f32, kind="ExternalInput")
    d_yblk = nc.dram_tensor("y_blocks", [128, N_BLK, N_CLASS], f32, kind="ExternalInput")
    d_ublk = nc.dram_tensor("u_blocks", [128, N_BLK, BIT], f32, kind="ExternalInput")
    d_vt = nc.dram_tensor("VT", [N_CLASS, BIT], f32, kind="ExternalInput")
    d_i100 = nc.dram_tensor("I100", [N_CLASS, N_CLASS], f32, kind="ExternalInput")
    d_j3 = nc.dram_tensor("J3", [N_CLASS, 3, N_CLASS], f32, kind="ExternalInput")

    d_partials = nc.dram_tensor("partials", [128, 2], f32, kind="ExternalOutput")
    d_vout = nc.dram_tensor("V_out", [BIT, N_CLASS], f32, kind="ExternalOutput")

    with tile.TileContext(nc) as tc:
        with (
            tc.tile_pool(name="cpool", bufs=1) as cpool,
            tc.tile_pool(name="wpool", bufs=2) as wpool,
            tc.tile_pool(name="mpsum", bufs=2, space="PSUM") as mpsum,
            tc.tile_pool(name="gpsum", bufs=1, space="PSUM") as gpsum,
        ):
            # ---- constants into SBUF ----
            c_uhalf = cpool.tile([BIT, BATCH], bf16, name="c_uhalf")
            c_ytaug = cpool.tile([N_CLASS + 1, BATCH], bf16, name="c_ytaug")
            c_ytp = cpool.tile([N_CLASS, BATCH], f32, name="c_ytp")
            c_yblk = cpool.tile([128, N_BLK, N_CLASS], f32, name="c_yblk")
            c_ublk = cpool.tile([128, N_BLK, BIT], f32, name="c_ublk")
            c_vt = cpool.tile([N_CLASS, BIT], f32, name="c_vt")
            c_i100 = cpool.tile([N_CLASS, N_CLASS], f32, name="c_i100")
            c_j3 = cpool.tile([N_CLASS, 3, N_CLASS], f32, name="c_j3")
            nc.sync.dma_start(c_uhalf[:], d_uhalf[:])
            nc.sync.dma_start(c_ytaug[:], d_ytaug[:])
            nc.sync.dma_start(c_ytp[:], d_ytp[:])
            nc.sync.dma_start(c_yblk[:], d_yblk[:])
            nc.sync.dma_start(c_ublk[:], d_ublk[:])
            nc.sync.dma_start(c_vt[:], d_vt[:])
            nc.sync.dma_start(c_i100[:], d_i100[:])
            nc.sync.dma_start(c_j3[:], d_j3[:])

            ones128 = cpool.tile([128, 1], f32, name="ones128")
            nc.vector.memset(ones128[:], 1.0)

            acc_m = cpool.tile([128, N_CHUNKS * N_BLK], f32, name="acc_m")
            acc_q = cpool.tile([128, N_BLK], f32, name="acc_q")
            nc.vector.memset(acc_m[:], 0.0)
            nc.vector.memset(acc_q[:], 0.0)
            bt_all = cpool.tile([128, N_BLK, BIT], f32, name="bt_all")
            ct = cpool.tile([N_CLASS, BIT], f32, name="ct")
            bts = cpool.tile([N_CLASS, 3, BIT], f32, name="bts")
            kv = cpool.tile([N_CLASS, 3], f32, name="kv")

            # ---- prologue: b, quant partials, Bt, counts ----
            nc.scalar.activation(ct[:], c_vt[:], Act.Sign)  # sign(V).T
            if not ENABLE_PROLOGUE:
                nc.vector.memset(bt_all[:], 0.0)
            for blk in range(N_BLK if ENABLE_PROLOGUE else 0):
                pb = gpsum.tile([128, BIT], f32, name="pb", tag="cgP")
                nc.tensor.matmul(
                    pb[:], c_ytp[:, blk * 128:(blk + 1) * 128], ct[:],
                    start=True, stop=True,
                )
                bpre = wpool.tile([128, BIT], f32, name="bpre", tag="bpre")
                nc.vector.tensor_tensor(bpre[:], pb[:], c_ublk[:, blk, :], Alu.add)
                nc.scalar.activation(bt_all[:, blk, :], bpre[:], Act.Sign)
                dq = wpool.tile([128, BIT], f32, name="dq", tag="dq")
                nc.vector.tensor_tensor(dq[:], bt_all[:, blk, :], c_ublk[:, blk, :], Alu.subtract)
                # (tensor_tensor_reduce crashes this runtime; Square+accum on ACT)
                dqj = wpool.tile([128, BIT], f32, name="dqj", tag="dqj")
                nc.scalar.activation(dqj[:], dq[:], Act.Square,
                                     accum_out=acc_q[:, blk:blk + 1])

            btp = gpsum.tile([N_CLASS, BIT], f32, name="btp", tag="cgB")
            for blk in range(N_BLK):
                nc.tensor.matmul(
                    btp[:], c_yblk[:, blk, :], bt_all[:, blk, :],
                    start=(blk == 0), stop=(blk == N_BLK - 1),
                )
            cntp = gpsum.tile([N_CLASS, 1], f32, name="cntp", tag="cgF")
            for blk in range(N_BLK):
                nc.tensor.matmul(
                    cntp[:], c_yblk[:, blk, :], ones128[:],
                    start=(blk == 0), stop=(blk == N_BLK - 1),
                )
            for a, alpha in enumerate(ALPHAS):
                nc.vector.tensor_scalar(
                    bts[:, a, :], btp[:], float(alpha * C1), float(-alpha * C3),
                    Alu.mult, Alu.add,
                )
                nc.vector.tensor_scalar(
                    kv[:, a:a + 1], cntp[:], float(-alpha * C1),
                    float(1.0 - alpha * C3 + 128 * alpha * C2),
                    Alu.mult, Alu.add,
                )

            # ---- center gradient: 200 iterations on W = V.T [100, 64] ----
            cg_scope = nc.named_scope("cg")
            cg_scope.__enter__()
            W = c_vt
            for i in range(CG_ITERS if ENABLE_CG else 0):
                a = _alpha_idx(i)
                alpha = ALPHAS[a]
                beta = -alpha * C2
                sp = wpool.tile([N_CLASS, BIT], f32, name="sp", tag="sp")
                nc.vector.tensor_scalar(
                    sp[:], W[:], 0.0, float(2 * alpha * C3), Alu.is_gt, Alu.mult,
                )
                x = wpool.tile([N_CLASS, BIT], f32, name="x", tag="x")
                nc.vector.tensor_tensor(x[:], sp[:], bts[:, a, :], Alu.add)

                fps = gpsum.tile([BIT, BIT], f32, name="fps", tag="cgF")
                nc.tensor.matmul(fps[:], W[:], W[:], start=True, stop=True)
                fb = wpool.tile([BIT, BIT], f32, name="fb", tag="fb")
                nc.vector.tensor_scalar(fb[:], fps[:], float(beta), None, Alu.mult)

                ptp = gpsum.tile([BIT, N_CLASS], f32, name="ptp", tag="cgT")
                nc.tensor.transpose(ptp[:], W[:], c_i100[:])
                vc = wpool.tile([BIT, N_CLASS], f32, name="vc", tag="vc")
                nc.vector.tensor_copy(vc[:], ptp[:])

                pp = gpsum.tile([N_CLASS, BIT], f32, name="pp", tag="cgP")
                nc.tensor.matmul(pp[:], vc[:], fb[:], start=True, stop=False)
                nc.tensor.matmul(pp[:], c_j3[:, a, :], W[:], start=False, stop=False)
                nc.tensor.matmul(pp[:], c_i100[:], x[:], start=False, stop=True)

                wn = wpool.tile([N_CLASS, BIT], f32, name="wn", tag="W")
                nc.vector.scalar_tensor_tensor(
                    out=wn[:], in0=W[:], scalar=kv[:, a:a + 1], in1=pp[:],
                    op0=Alu.mult, op1=Alu.add,
                )
                W = wn

            vtp = gpsum.tile([BIT, N_CLASS], f32, name="vtp", tag="cgT")
            nc.tensor.transpose(vtp[:], W[:], c_i100[:])
            vout = wpool.tile([BIT, N_CLASS], f32, name="vout", tag="vout")
            nc.vector.tensor_copy(vout[:], vtp[:])
            nc.sync.dma_start(d_vout[:], vout[:])
            cg_scope.__exit__(None, None, None)

            # ---- metric loss over this core's shard ----
            m_scope = nc.named_scope("metric")
            m_scope.__enter__()
            for k in range(N_CHUNKS if ENABLE_METRIC else 0):
                uc = wpool.tile([BIT, CHUNK], bf16, name="uc", tag="uc")
                nc.sync.dma_start(uc[:], d_us[:, k * CHUNK:(k + 1) * CHUNK])
                yc = wpool.tile([N_CLASS + 1, CHUNK], bf16, name="yc", tag="yc")
                nc.sync.dma_start(yc[:], d_ys[:, k * CHUNK:(k + 1) * CHUNK])
                for blk in range(N_BLK):
                    tbig = wpool.tile([128, CHUNK], f32, name="tbig", tag="t")
                    spo = wpool.tile([128, CHUNK], f32, name="spo", tag="spo")
                    for j in range(N_SUBS):
                        ipp = mpsum.tile([128, SUB], f32, name="ipp", tag="ip")
                        nc.tensor.matmul(
                            ipp[:], c_uhalf[:, blk * 128:(blk + 1) * 128],
                            uc[:, j * SUB:(j + 1) * SUB], start=True, stop=True,
                        )
                        rp = mpsum.tile([128, SUB], f32, name="rp", tag="r")
                        nc.tensor.matmul(
                            rp[:], c_ytaug[:, blk * 128:(blk + 1) * 128],
                            yc[:, j * SUB:(j + 1) * SUB], start=True, stop=True,
                        )
                        # DVE can read only one PSUM operand per instruction:
                        # stage r in SBUF, then multiply with ip from PSUM.
                        rsb = wpool.tile([128, SUB], f32, name="rsb", tag="rsb")
                        nc.vector.tensor_copy(rsb[:], rp[:])
                        nc.vector.tensor_tensor(
                            tbig[:, j * SUB:(j + 1) * SUB], ipp[:], rsb[:], Alu.mult,
                        )
                    # softplus(t) = ln(1 + e^t); t <= ~51 so e^t stays finite
                    # in fp32. Exp and Ln live in one ACT table set
                    # (natural_log_exp_and_others) -> no table switching.
                    col = k * N_BLK + blk
                    etile = wpool.tile([128, CHUNK], f32, name="etile", tag="et")
                    nc.scalar.activation(
                        etile[:], tbig[:], Act.Exp, bias=float(M), scale=1.0,
                    )
                    nc.scalar.activation(
                        spo[:], etile[:], Act.Ln, bias=1.0, scale=1.0,
                        accum_out=acc_m[:, col:col + 1],
                    )

            m_scope.__exit__(None, None, None)

            # ---- reductions + store ----
            pr = wpool.tile([128, 2], f32, name="pr", tag="pr")
            nc.vector.tensor_reduce(pr[:, 0:1], acc_m[:], mybir.AxisListType.X, Alu.add)
            nc.vector.tensor_reduce(pr[:, 1:2], acc_q[:], mybir.AxisListType.X, Alu.add)
            nc.sync.dma_start(d_partials[:], pr[:])

    nc.compile()
    return nc


def _host_inputs(u, y, ind, U, Y, V):
    """Scatter + shard + layout prep. Returns per-core input maps."""
    u = np.asarray(u, np.float32)
    y = np.asarray(y, np.float32)
    ind = np.asarray(ind)
    U2 = np.array(U, np.float32, copy=True)
    Y2 = np.array(Y, np.float32, copy=True)
    U2[:, ind] = u.T
    Y2[:, ind] = y.T

    u_half = np.ascontiguousarray(0.5 * u.T).astype(BF16)
    yt_aug = np.concatenate(
        [-2.0 * y.T, np.ones((1, BATCH), np.float32)], axis=0
    ).astype(BF16)
    yt_plain = np.ascontiguousarray(y.T)
    y_blocks = np.ascontiguousarray(
        y.reshape(N_BLK, 128, N_CLASS).transpose(1, 0, 2))
    u_blocks = np.ascontiguousarray(
        u.reshape(N_BLK, 128, BIT).transpose(1, 0, 2))
    VT = np.ascontiguousarray(np.asarray(V, np.float32).T)
    I100 = np.eye(N_CLASS, dtype=np.float32)
    J3 = np.stack(
        [np.full((N_CLASS, N_CLASS), 64.0 * (-a * C2), np.float32) for a in ALPHAS],
        axis=1,
    )
    ones_row = np.ones((1, NSH), np.float32)

    shared = dict(
        u_half=u_half, yt_aug=yt_aug, yt_plain=yt_plain, y_blocks=y_blocks,
        u_blocks=u_blocks, VT=VT, I100=I100, J3=J3,
    )
    in_maps = []
    for c in range(N_CORES):
        sl = slice(c * NSH, (c + 1) * NSH)
        in_maps.append(dict(
            shared,
            Us=np.ascontiguousarray(U2[:, sl]).astype(BF16),
            Ys=np.concatenate([Y2[:, sl], ones_row], axis=0).astype(BF16),
        ))
    return in_maps


_PROGRAM = None


def kernel(u, y, ind, U, Y, V):
    global _PROGRAM, LAST_RESULTS
    from concourse.bass_utils import run_bass_kernel_spmd

    in_maps = _host_inputs(u, y, ind, U, Y, V)
    if _PROGRAM is None:
        _PROGRAM = _build_program()
    res = run_bass_kernel_spmd(
        _PROGRAM, in_maps, core_ids=list(range(N_CORES)), trace=TRACE,
    )
    LAST_RESULTS = res

    msum = 0.0
    for c in range(N_CORES):
        msum += float(res.results[c]["partials"][:, 0].astype(np.float64).sum())
    qsum = float(res.results[0]["partials"][:, 1].astype(np.float64).sum())
    metric = msum / (BATCH * NUM_TRAIN)
    quant = qsum / (BIT * BATCH)
    loss = np.float32(metric + ETA * quant)
    V_new = np.asarray(res.results[0]["V_out"], np.float32)
    return loss, V_new


# revision 23
# speedup vs baseline: 411.5469x; 1.2722x over previous
"""DFH loss kernel for Trainium2, data-parallel across 8 NeuronCores.

Computation (see reference):
  U[:, ind] = u.T; Y[:, ind] = y.T                       (host, part of sharding prep)
  b = sign(C @ y.T + u.T), C = sign(V)                   (device prologue)
  V_new = 200 GD steps on V                              (device, redundant per core)
  metric = mean(softplus(M + (1-2s) * 0.5*(u@U)))        (device, sharded over num_train)
  quant  = mean((b - u.T)^2)
  loss = metric + ETA * quant

Sharding: U/Y split along num_train (12500 cols/core); everything else replicated.
Per-core partial sums are combined on the host (8 scalars).

Device-side math notes:
  s = y@Y is exactly {0,1} (one-hot x one-hot), so the two softplus branches
  collapse to softplus(M + r*ip) with r = 1-2s. r comes from one K=101 bf16
  matmul with lhsT = [-2*y.T; 1] and rhs = [Y; 1] (the +1 fold).
  The clip(ip, -100, 50) never binds for this data distribution (max |ip| < 50
  checked against the reference; softplus is evaluated with proper asymptotics
  on ACT, so no overflow concerns).

  Center_gradient is run in transposed form W = V.T [100, 64]:
    W <- kv ⊙ W + beta*(W @ (W.T W)) + 64*beta*(J @ W) + alpha*c3*S' + Bts
  with S' = 2*(W>0), kv/Bts/J folding all per-alpha constants.
"""

import sys
import numpy as np

for _p in ("/opt/trn_rl_repo", "/root/.axon_site/_ro/trn_rl_repo"):
    if _p not in sys.path:
        sys.path.append(_p)

import ml_dtypes

BF16 = ml_dtypes.bfloat16

BIT = 64
N_CLASS = 100
NUM_TRAIN = 100000
BATCH = 512
MU = 1.0
M = 1.0
ETA = 0.5
VUL = 1.0
NTA = 1.0

N_CORES = 8
NSH = NUM_TRAIN // N_CORES          # 12500 columns per core
CHUNK = 2500                        # DMA/ACT chunk (cols)
SUB = 500                           # matmul N (one PSUM bank)
N_CHUNKS = NSH // CHUNK
N_SUBS = CHUNK // SUB
N_BLK = BATCH // 128                # 4 batch blocks

C1 = 2.0 / (BIT * BATCH)
C2 = 4.0 / (N_CLASS * N_CLASS) * VUL
C3 = 2.0 / (BIT * N_CLASS) * NTA
ALPHAS = (0.03, 0.003, 0.0003)
N_ITERS = 200

TRACE = False           # set by test.py to capture an NTFF profile
LAST_RESULTS = None     # BassKernelResults of the most recent run
ENABLE_CG = True        # debug bisect toggles
ENABLE_METRIC = True
ENABLE_PROLOGUE = True
CG_ITERS = N_ITERS


def _alpha_idx(i):
    return 0 if i < 149 else (1 if i < 179 else 2)


def _build_program():
    import concourse.bacc as bacc
    import concourse.mybir as mybir
    from concourse import tile

    dt = mybir.dt
    f32 = dt.float32
    bf16 = dt.bfloat16
    Alu = mybir.AluOpType
    Act = mybir.ActivationFunctionType

    nc = bacc.Bacc("TRN2", target_bir_lowering=False, debug=False)

    # ---- DRAM I/O ----
    d_uhalf = nc.dram_tensor("u_half", [BIT, BATCH], bf16, kind="ExternalInput")
    d_ytaug = nc.dram_tensor("yt_aug", [N_CLASS + 1, BATCH], bf16, kind="ExternalInput")
    d_us = nc.dram_tensor("Us", [BIT, NSH], bf16, kind="ExternalInput")
    d_ys = nc.dram_tensor("Ys", [N_CLASS + 1, NSH], bf16, kind="ExternalInput")
    d_ytp = nc.dram_tensor("yt_plain", [N_CLASS, BATCH], f32, kind="ExternalInput")
    d_yblk = nc.dram_tensor("y_blocks", [128, N_BLK, N_CLASS], f32, kind="ExternalInput")
    d_ublk = nc.dram_tensor("u_blocks", [128, N_BLK, BIT], f32, kind="ExternalInput")
    d_vt = nc.dram_tensor("VT", [N_CLASS, BIT], f32, kind="ExternalInput")
    d_i100 = nc.dram_tensor("I100", [N_CLASS, N_CLASS], f32, kind="ExternalInput")
    d_j3 = nc.dram_tensor("J3", [N_CLASS, 3, N_CLASS], f32, kind="ExternalInput")

    d_partials = nc.dram_tensor("partials", [128, 2], f32, kind="ExternalOutput")
    d_vout = nc.dram_tensor("V_out", [BIT, N_CLASS], f32, kind="ExternalOutput")

    with tile.TileContext(nc) as tc:
        with (
            tc.tile_pool(name="cpool", bufs=1) as cpool,
            tc.tile_pool(name="wpool", bufs=2) as wpool,
            tc.tile_pool(name="mpsum", bufs=2, space="PSUM") as mpsum,
            tc.tile_pool(name="gpsum", bufs=1, space="PSUM") as gpsum,
        ):
            # ---- constants into SBUF ----
            c_uhalf = cpool.tile([BIT, BATCH], bf16, name="c_uhalf")
            c_ytaug = cpool.tile([N_CLASS + 1, BATCH], bf16, name="c_ytaug")
            c_ytp = cpool.tile([N_CLASS, BATCH], f32, name="c_ytp")
            c_yblk = cpool.tile([128, N_BLK, N_CLASS], f32, name="c_yblk")
            c_ublk = cpool.tile([128, N_BLK, BIT], f32, name="c_ublk")
            c_vt = cpool.tile([N_CLASS, BIT], f32, name="c_vt")
            c_i100 = cpool.tile([N_CLASS, N_CLASS], f32, name="c_i100")
            c_j3 = cpool.tile([N_CLASS, 3, N_CLASS], f32, name="c_j3")
            nc.sync.dma_start(c_uhalf[:], d_uhalf[:])
            nc.sync.dma_start(c_ytaug[:], d_ytaug[:])
            nc.sync.dma_start(c_ytp[:], d_ytp[:])
            nc.sync.dma_start(c_yblk[:], d_yblk[:])
            nc.sync.dma_start(c_ublk[:], d_ublk[:])
            nc.sync.dma_start(c_vt[:], d_vt[:])
            nc.sync.dma_start(c_i100[:], d_i100[:])
            nc.sync.dma_start(c_j3[:], d_j3[:])

            ones128 = cpool.tile([128, 1], f32, name="ones128")
            nc.vector.memset(ones128[:], 1.0)

            acc_m = cpool.tile([128, N_CHUNKS * N_BLK], f32, name="acc_m")
            acc_q = cpool.tile([128, N_BLK], f32, name="acc_q")
            nc.vector.memset(acc_m[:], 0.0)
            nc.vector.memset(acc_q[:], 0.0)
            bt_all = cpool.tile([128, N_BLK, BIT], f32, name="bt_all")
            ct = cpool.tile([N_CLASS, BIT], f32, name="ct")
            bts = cpool.tile([N_CLASS, 3, BIT], f32, name="bts")
            kv = cpool.tile([N_CLASS, 3], f32, name="kv")

            # ---- prologue: b, quant partials, Bt, counts ----
            nc.scalar.activation(ct[:], c_vt[:], Act.Sign)  # sign(V).T
            if not ENABLE_PROLOGUE:
                nc.vector.memset(bt_all[:], 0.0)
            for blk in range(N_BLK if ENABLE_PROLOGUE else 0):
                pb = gpsum.tile([128, BIT], f32, name="pb", tag="cgP")
                nc.tensor.matmul(
                    pb[:], c_ytp[:, blk * 128:(blk + 1) * 128], ct[:],
                    start=True, stop=True,
                )
                bpre = wpool.tile([128, BIT], f32, name="bpre", tag="bpre")
                nc.vector.tensor_tensor(bpre[:], pb[:], c_ublk[:, blk, :], Alu.add)
                nc.scalar.activation(bt_all[:, blk, :], bpre[:], Act.Sign)
                dq = wpool.tile([128, BIT], f32, name="dq", tag="dq")
                nc.vector.tensor_tensor(dq[:], bt_all[:, blk, :], c_ublk[:, blk, :], Alu.subtract)
                # (tensor_tensor_reduce crashes this runtime; Square+accum on ACT)
                dqj = wpool.tile([128, BIT], f32, name="dqj", tag="dqj")
                nc.scalar.activation(dqj[:], dq[:], Act.Square,
                                     accum_out=acc_q[:, blk:blk + 1])

            btp = gpsum.tile([N_CLASS, BIT], f32, name="btp", tag="cgB")
            for blk in range(N_BLK):
                nc.tensor.matmul(
                    btp[:], c_yblk[:, blk, :], bt_all[:, blk, :],
                    start=(blk == 0), stop=(blk == N_BLK - 1),
                )
            cntp = gpsum.tile([N_CLASS, 1], f32, name="cntp", tag="cgF")
            for blk in range(N_BLK):
                nc.tensor.matmul(
                    cntp[:], c_yblk[:, blk, :], ones128[:],
                    start=(blk == 0), stop=(blk == N_BLK - 1),
                )
            for a, alpha in enumerate(ALPHAS):
                nc.vector.tensor_scalar(
                    bts[:, a, :], btp[:], float(alpha * C1), float(-alpha * C3),
                    Alu.mult, Alu.add,
                )
                nc.vector.tensor_scalar(
                    kv[:, a:a + 1], cntp[:], float(-alpha * C1),
                    float(1.0 - alpha * C3 + 128 * alpha * C2),
                    Alu.mult, Alu.add,
                )

            # ---- center gradient: 200 iterations on W = V.T [100, 64] ----
            cg_scope = nc.named_scope("cg")
            cg_scope.__enter__()
            W = c_vt
            for i in range(CG_ITERS if ENABLE_CG else 0):
                a = _alpha_idx(i)
                alpha = ALPHAS[a]
                beta = -alpha * C2
                sp = wpool.tile([N_CLASS, BIT], f32, name="sp", tag="sp")
                nc.vector.tensor_scalar(
                    sp[:], W[:], 0.0, float(2 * alpha * C3), Alu.is_gt, Alu.mult,
                )
                x = wpool.tile([N_CLASS, BIT], f32, name="x", tag="x")
                nc.vector.tensor_tensor(x[:], sp[:], bts[:, a, :], Alu.add)

                fps = gpsum.tile([BIT, BIT], f32, name="fps", tag="cgF")
                nc.tensor.matmul(fps[:], W[:], W[:], start=True, stop=True)
                fb = wpool.tile([BIT, BIT], f32, name="fb", tag="fb")
                nc.vector.tensor_scalar(fb[:], fps[:], float(beta), None, Alu.mult)

                ptp = gpsum.tile([BIT, N_CLASS], f32, name="ptp", tag="cgT")
                nc.tensor.transpose(ptp[:], W[:], c_i100[:])
                vc = wpool.tile([BIT, N_CLASS], f32, name="vc", tag="vc")
                nc.vector.tensor_copy(vc[:], ptp[:])

                pp = gpsum.tile([N_CLASS, BIT], f32, name="pp", tag="cgP")
                nc.tensor.matmul(pp[:], vc[:], fb[:], start=True, stop=False)
                nc.tensor.matmul(pp[:], c_j3[:, a, :], W[:], start=False, stop=False)
                nc.tensor.matmul(pp[:], c_i100[:], x[:], start=False, stop=True)

                wn = wpool.tile([N_CLASS, BIT], f32, name="wn", tag="W")
                nc.vector.scalar_tensor_tensor(
                    out=wn[:], in0=W[:], scalar=kv[:, a:a + 1], in1=pp[:],
                    op0=Alu.mult, op1=Alu.add,
                )
                W = wn

            vtp = gpsum.tile([BIT, N_CLASS], f32, name="vtp", tag="cgT")
            nc.tensor.transpose(vtp[:], W[:], c_i100[:])
            vout = wpool.tile([BIT, N_CLASS], f32, name="vout", tag="vout")
            nc.vector.tensor_copy(vout[:], vtp[:])
            nc.sync.dma_start(d_vout[:], vout[:])
            cg_scope.__exit__(None, None, None)

            # ---- metric loss over this core's shard ----
            m_scope = nc.named_scope("metric")
            m_scope.__enter__()
            for k in range(N_CHUNKS if ENABLE_METRIC else 0):
                uc = wpool.tile([BIT, CHUNK], bf16, name="uc", tag="uc")
                nc.sync.dma_start(uc[:], d_us[:, k * CHUNK:(k + 1) * CHUNK])
                yc = wpool.tile([N_CLASS + 1, CHUNK], bf16, name="yc", tag="yc")
                nc.sync.dma_start(yc[:], d_ys[:, k * CHUNK:(k + 1) * CHUNK])
                for blk in range(N_BLK):
                    tbig = wpool.tile([128, CHUNK], f32, name="tbig", tag="t")
                    spo = wpool.tile([128, CHUNK], f32, name="spo", tag="spo")
                    for j in range(N_SUBS):
                        ipp = mpsum.tile([128, SUB], f32, name="ipp", tag="ip")
                        nc.tensor.matmul(
                            ipp[:], c_uhalf[:, blk * 128:(blk + 1) * 128],
                            uc[:, j * SUB:(j + 1) * SUB], start=True, stop=True,
                        )
                        rp = mpsum.tile([128, SUB], f32, name="rp", tag="r")
                        nc.tensor.matmul(
                            rp[:], c_ytaug[:, blk * 128:(blk + 1) * 128],
                            yc[:, j * SUB:(j + 1) * SUB], start=True, stop=True,
                        )
                        # DVE can read only one PSUM operand per instruction:
                        # stage r in SBUF, then multiply with ip from PSUM.
                        # The copy runs on ACT (ScE is next to PSUM and ~75%
                        # idle) so it stops colliding with the CG chain's DVE
                        # ops — the timeline showed these 646ns copies
                        # stretching every CG iteration.
                        rsb = wpool.tile([128, SUB], f32, name="rsb", tag="rsb")
                        nc.scalar.copy(rsb[:], rp[:])
                        nc.vector.tensor_tensor(
                            tbig[:, j * SUB:(j + 1) * SUB], ipp[:], rsb[:], Alu.mult,
                        )
                    # softplus(t) = ln(1 + e^t); t <= ~51 so e^t stays finite
                    # in fp32. Exp and Ln live in one ACT table set
                    # (natural_log_exp_and_others) -> no table switching.
                    col = k * N_BLK + blk
                    etile = wpool.tile([128, CHUNK], f32, name="etile", tag="et")
                    nc.scalar.activation(
                        etile[:], tbig[:], Act.Exp, bias=float(M), scale=1.0,
                    )
                    nc.scalar.activation(
                        spo[:], etile[:], Act.Ln, bias=1.0, scale=1.0,
                        accum_out=acc_m[:, col:col + 1],
                    )

            m_scope.__exit__(None, None, None)

            # ---- reductions + store ----
            pr = wpool.tile([128, 2], f32, name="pr", tag="pr")
            nc.vector.tensor_reduce(pr[:, 0:1], acc_m[:], mybir.AxisListType.X, Alu.add)
            nc.vector.tensor_reduce(pr[:, 1:2], acc_q[:], mybir.AxisListType.X, Alu.add)
            nc.sync.dma_start(d_partials[:], pr[:])

    nc.compile()
    return nc


def _host_inputs(u, y, ind, U, Y, V):
    """Scatter + shard + layout prep. Returns per-core input maps."""
    u = np.asarray(u, np.float32)
    y = np.asarray(y, np.float32)
    ind = np.asarray(ind)
    U2 = np.array(U, np.float32, copy=True)
    Y2 = np.array(Y, np.float32, copy=True)
    U2[:, ind] = u.T
    Y2[:, ind] = y.T

    u_half = np.ascontiguousarray(0.5 * u.T).astype(BF16)
    yt_aug = np.concatenate(
        [-2.0 * y.T, np.ones((1, BATCH), np.float32)], axis=0
    ).astype(BF16)
    yt_plain = np.ascontiguousarray(y.T)
    y_blocks = np.ascontiguousarray(
        y.reshape(N_BLK, 128, N_CLASS).transpose(1, 0, 2))
    u_blocks = np.ascontiguousarray(
        u.reshape(N_BLK, 128, BIT).transpose(1, 0, 2))
    VT = np.ascontiguousarray(np.asarray(V, np.float32).T)
    I100 = np.eye(N_CLASS, dtype=np.float32)
    J3 = np.stack(
        [np.full((N_CLASS, N_CLASS), 64.0 * (-a * C2), np.float32) for a in ALPHAS],
        axis=1,
    )
    ones_row = np.ones((1, NSH), np.float32)

    shared = dict(
        u_half=u_half, yt_aug=yt_aug, yt_plain=yt_plain, y_blocks=y_blocks,
        u_blocks=u_blocks, VT=VT, I100=I100, J3=J3,
    )
    in_maps = []
    for c in range(N_CORES):
        sl = slice(c * NSH, (c + 1) * NSH)
        in_maps.append(dict(
            shared,
            Us=np.ascontiguousarray(U2[:, sl]).astype(BF16),
            Ys=np.concatenate([Y2[:, sl], ones_row], axis=0).astype(BF16),
        ))
    return in_maps


_PROGRAM = None


def kernel(u, y, ind, U, Y, V):
    global _PROGRAM, LAST_RESULTS
    from concourse.bass_utils import run_bass_kernel_spmd

    in_maps = _host_inputs(u, y, ind, U, Y, V)
    if _PROGRAM is None:
        _PROGRAM = _build_program()
    res = run_bass_kernel_spmd(
        _PROGRAM, in_maps, core_ids=list(range(N_CORES)), trace=TRACE,
    )
    LAST_RESULTS = res

    msum = 0.0
    for c in range(N_CORES):
        msum += float(res.results[c]["partials"][:, 0].astype(np.float64).sum())
    qsum = float(res.results[0]["partials"][:, 1].astype(np.float64).sum())
    metric = msum / (BATCH * NUM_TRAIN)
    quant = qsum / (BIT * BATCH)
    loss = np.float32(metric + ETA * quant)
    V_new = np.asarray(res.results[0]["V_out"], np.float32)
    return loss, V_new
